# revision 1
# baseline (speedup 1.0000x reference)
"""Bass/Trainium2 kernel for nn_CrossAttentionLayer.

out = softmax((x_q Wq^T + bq)(x_k Wk^T + bk)^T) (x_v Wv^T + bv)

Sharding: data-parallel over batch B=8 across the 8 NeuronCores.
Exact math simplifications used:
  - bk drops out of softmax (adds a per-row constant to the logits).
  - bv is added on the host (softmax rows sum to 1, so attn @ (v0 + bv)
    = attn @ v0 + bv).
  - softmax normalization (divide by row-sum) commutes with the PV
    matmul, so the device returns the unnormalized PV product plus
    row-sums and the host divides.
Device layout: scores are computed TRANSPOSED ([key, query] tiles) so
the PV matmul can consume v in its natural [key, d] layout with no
on-chip transpose of the attention matrix; row-sums over the key
(partition) axis are computed with a ones-vector matmul on the PE.
"""

import sys

if "/opt/trn_rl_repo" not in sys.path:
    sys.path.insert(0, "/opt/trn_rl_repo")

import numpy as np

B = 8          # batch == number of cores
D = 1024       # model/latent dim
N = 2048       # tokens (queries == keys)
P = 128        # partitions
DC = D // P    # 8 chunks of the d/e axis
JT = N // P    # 16 key tiles
F = 512        # matmul moving free dim (fp32 max)
NB = N // F    # 4 query blocks

_CACHE = {}


def _build_nc():
    import concourse.bass as bass
    import concourse.mybir as mybir
    import concourse.tile as tile
    from concourse import bacc
    from concourse.masks import make_identity
    from contextlib import ExitStack

    f32 = mybir.dt.float32
    f32r = mybir.dt.float32r
    EXP = mybir.ActivationFunctionType.Exp

    nc = bacc.Bacc("TRN2", target_bir_lowering=False, debug=False, num_devices=B)

    xqt = nc.dram_tensor("xqt", [D, N], f32r, kind="ExternalInput").ap()
    xkt = nc.dram_tensor("xkt", [D, N], f32r, kind="ExternalInput").ap()
    xvt = nc.dram_tensor("xvt", [D, N], f32r, kind="ExternalInput").ap()
    wqt = nc.dram_tensor("wqt", [D, D], f32r, kind="ExternalInput").ap()
    wkt = nc.dram_tensor("wkt", [D, D], f32r, kind="ExternalInput").ap()
    wvt = nc.dram_tensor("wvt", [D, D], f32r, kind="ExternalInput").ap()
    bqt = nc.dram_tensor("bqt", [P, DC], f32, kind="ExternalInput").ap()

    v_int = nc.dram_tensor("v_int", [N, D], f32r).ap()
    qt_int = nc.dram_tensor("qt_int", [D, N], f32r).ap()

    acct = nc.dram_tensor("acct", [D, N], f32, kind="ExternalOutput").ap()
    rowsum = nc.dram_tensor("rowsum", [NB, F], f32, kind="ExternalOutput").ap()

    with ExitStack() as ctx:
        tc = ctx.enter_context(tile.TileContext(nc))
        big = ctx.enter_context(tc.tile_pool(name="big", bufs=2))
        ktp = ctx.enter_context(tc.tile_pool(name="ktp", bufs=1))
        xjp = ctx.enter_context(tc.tile_pool(name="xjp", bufs=3))
        natp = ctx.enter_context(tc.tile_pool(name="natp", bufs=1))
        stp = ctx.enter_context(tc.tile_pool(name="stp", bufs=2))
        qsp = ctx.enter_context(tc.tile_pool(name="qsp", bufs=2))
        vip = ctx.enter_context(tc.tile_pool(name="vip", bufs=2))
        cst = ctx.enter_context(tc.tile_pool(name="cst", bufs=1))
        psa = ctx.enter_context(tc.tile_pool(name="psa", bufs=3, space="PSUM"))
        pst = ctx.enter_context(tc.tile_pool(name="pst", bufs=2, space="PSUM"))
        psm = ctx.enter_context(tc.tile_pool(name="psm", bufs=2, space="PSUM"))
        psr = ctx.enter_context(tc.tile_pool(name="psr", bufs=1, space="PSUM"))

        ident = cst.tile([P, P], f32, name="ident", tag="c_id")
        make_identity(nc, ident)
        ones_f = cst.tile([P, 1], f32, name="ones_f", tag="c_onesf")
        nc.vector.memset(ones_f, 1.0)
        ones = cst.tile([P, 1], f32r, name="ones", tag="c_ones")
        nc.vector.tensor_copy(ones, ones_f)
        bq_sb = cst.tile([P, DC], f32, name="bq_sb", tag="c_bq")

        kt_sb = ktp.tile([P, DC, N], f32r, name="kt_sb", tag="kt")

        def load_w(dst, src):
            srcr = src.rearrange("(c p) e -> p c e", p=P)
            for c in range(DC):
                nc.sync.dma_start(out=dst[:, c, :], in_=srcr[:, c, :])

        xkr = xkt.rearrange("(c p) n -> p c n", p=P)
        xt0 = xjp.tile([P, DC, P], f32r, name="xt_k", tag="xj")
        nc.sync.dma_start(out=xt0, in_=xkr[:, :, 0:P])
        wk_sb = big.tile([P, DC, D], f32r, name="wk_sb", tag="big")
        load_w(wk_sb, wkt)
        wv_sb = big.tile([P, DC, D], f32r, name="wv_sb", tag="big")

        # ---- Phase 1a: kT[e, j] = (x_k Wk^T)^T, resident in SBUF ----
        for jt in range(JT):
            if jt == 0:
                xt = xt0
            else:
                xt = xjp.tile([P, DC, P], f32r, name="xt_k", tag="xj")
                nc.sync.dma_start(out=xt, in_=xkr[:, :, jt * P:(jt + 1) * P])
            knat = natp.tile([P, D], f32, name="knat", tag="nat")
            for nd in range(2):
                ps = psa.tile([P, F], f32, name="ps_k", tag="psa")
                for dp in range(DC):
                    nc.tensor.matmul(
                        ps,
                        lhsT=xt[:, dp, :],
                        rhs=wk_sb[:, dp, nd * F:(nd + 1) * F],
                        start=(dp == 0),
                        stop=(dp == DC - 1),
                    )
                nc.vector.tensor_copy(knat[:, nd * F:(nd + 1) * F], ps)
            for e in range(DC):
                pt = pst.tile([P, P], f32, name="pt_k", tag="pst")
                nc.tensor.transpose(pt, knat[:, e * P:(e + 1) * P], ident)
                nc.scalar.copy(kt_sb[:, e, jt * P:(jt + 1) * P], pt)

        # ---- Phase 1b: v = x_v Wv^T (no bias) -> DRAM ----
        load_w(wv_sb, wvt)
        xvr = xvt.rearrange("(c p) n -> p c n", p=P)
        for jt in range(JT):
            xt = xjp.tile([P, DC, P], f32r, name="xt_v", tag="xj")
            nc.sync.dma_start(out=xt, in_=xvr[:, :, jt * P:(jt + 1) * P])
            for nd in range(2):
                ps = psa.tile([P, F], f32, name="ps_v", tag="psa")
                for dp in range(DC):
                    nc.tensor.matmul(
                        ps,
                        lhsT=xt[:, dp, :],
                        rhs=wv_sb[:, dp, nd * F:(nd + 1) * F],
                        start=(dp == 0),
                        stop=(dp == DC - 1),
                    )
                st = stp.tile([P, F], f32r, name="st_v", tag="st")
                nc.vector.tensor_copy(st, ps)
                nc.sync.dma_start(
                    out=v_int[jt * P:(jt + 1) * P, nd * F:(nd + 1) * F], in_=st
                )

        # ---- Phase 1c: qT[e, i] = (x_q Wq^T + bq)^T -> DRAM ----
        wq_sb = big.tile([P, DC, D], f32r, name="wq_sb", tag="big")
        load_w(wq_sb, wqt)
        nc.sync.dma_start(out=bq_sb, in_=bqt)
        xqr = xqt.rearrange("(c p) n -> p c n", p=P)
        qtr = qt_int.rearrange("(c p) n -> p c n", p=P)
        for it in range(JT):
            xt = xjp.tile([P, DC, P], f32r, name="xt_q", tag="xj")
            nc.sync.dma_start(out=xt, in_=xqr[:, :, it * P:(it + 1) * P])
            qnat = natp.tile([P, D], f32, name="qnat", tag="nat")
            for nd in range(2):
                ps = psa.tile([P, F], f32, name="ps_q", tag="psa")
                for dp in range(DC):
                    nc.tensor.matmul(
                        ps,
                        lhsT=xt[:, dp, :],
                        rhs=wq_sb[:, dp, nd * F:(nd + 1) * F],
                        start=(dp == 0),
                        stop=(dp == DC - 1),
                    )
                nc.vector.tensor_copy(qnat[:, nd * F:(nd + 1) * F], ps)
            stq = stp.tile([P, DC, P], f32r, name="stq", tag="st")
            for e in range(DC):
                pt = pst.tile([P, P], f32, name="pt_q", tag="pst")
                nc.tensor.transpose(pt, qnat[:, e * P:(e + 1) * P], ident)
                nc.vector.tensor_scalar_add(stq[:, e, :], pt, bq_sb[:, e:e + 1])
            nc.sync.dma_start(out=qtr[:, :, it * P:(it + 1) * P], in_=stq)

        # ---- Phase 2: per 512-query block: scores^T, exp, rowsum, PV ----
        vr = v_int.rearrange("(t p) d -> p t d", p=P)
        for t in range(NB):
            qb = qsp.tile([P, DC, F], f32r, name="qb", tag="qs")
            for c in range(DC):
                nc.sync.dma_start(out=qb[:, c, :], in_=qtr[:, c, t * F:(t + 1) * F])
            ex = big.tile([P, JT, F], f32r, name="ex", tag="big")
            rp = psr.tile([1, F], f32, name="rp", tag="psr")
            for jt in range(JT):
                ps = psm.tile([P, F], f32, name="ps_s", tag="psm")
                for e in range(DC):
                    nc.tensor.matmul(
                        ps,
                        lhsT=kt_sb[:, e, jt * P:(jt + 1) * P],
                        rhs=qb[:, e, :],
                        start=(e == 0),
                        stop=(e == DC - 1),
                    )
                nc.scalar.activation(ex[:, jt, :], ps, EXP)
                nc.tensor.matmul(
                    rp,
                    lhsT=ones,
                    rhs=ex[:, jt, :],
                    start=(jt == 0),
                    stop=(jt == JT - 1),
                    skip_group_check=True,
                )
            rs = stp.tile([1, F], f32, name="rs", tag="strs")
            nc.vector.tensor_copy(rs, rp)
            nc.sync.dma_start(out=rowsum[t:t + 1, :], in_=rs)
            for d in range(DC):
                vd = vip.tile([P, JT, P], f32r, name="vd", tag="vi")
                nc.sync.dma_start(out=vd[:, :8, :], in_=vr[:, :8, d * P:(d + 1) * P])
                nc.sync.dma_start(out=vd[:, 8:, :], in_=vr[:, 8:, d * P:(d + 1) * P])
                pv = psa.tile([P, F], f32, name="pv", tag="psa")
                for jt in range(JT):
                    nc.tensor.matmul(
                        pv,
                        lhsT=vd[:, jt, :],
                        rhs=ex[:, jt, :],
                        start=(jt == 0),
                        stop=(jt == JT - 1),
                    )
                ot = stp.tile([P, F], f32, name="ot", tag="st")
                nc.vector.tensor_copy(ot, pv)
                nc.sync.dma_start(
                    out=acct[d * P:(d + 1) * P, t * F:(t + 1) * F], in_=ot
                )

    nc.compile()
    return nc


def get_nc():
    if "nc" not in _CACHE:
        _CACHE["nc"] = _build_nc()
    return _CACHE["nc"]


def make_in_maps(query, key, value, Wq, bq, Wk, bk, Wv, bv):
    query = np.asarray(query, dtype=np.float32)
    key = np.asarray(key, dtype=np.float32)
    value = np.asarray(value, dtype=np.float32)
    wqt = np.ascontiguousarray(np.asarray(Wq, dtype=np.float32).T)
    wkt = np.ascontiguousarray(np.asarray(Wk, dtype=np.float32).T)
    wvt = np.ascontiguousarray(np.asarray(Wv, dtype=np.float32).T)
    bqt = np.ascontiguousarray(np.asarray(bq, dtype=np.float32).reshape(DC, P).T)
    in_maps = []
    for b in range(B):
        in_maps.append(
            {
                "xqt": np.ascontiguousarray(query[b].T),
                "xkt": np.ascontiguousarray(key[b].T),
                "xvt": np.ascontiguousarray(value[b].T),
                "wqt": wqt,
                "wkt": wkt,
                "wvt": wvt,
                "bqt": bqt,
            }
        )
    return in_maps


def postprocess(results, bv):
    bv = np.asarray(bv, dtype=np.float32)
    outs = []
    for b in range(B):
        acct = results[b]["acct"]              # [D, N] unnormalized (attn@v)^T
        rsum = results[b]["rowsum"].reshape(N)  # [N] softmax denominators
        outs.append(acct.T / rsum[:, None] + bv[None, :])
    return np.stack(outs).astype(np.float32)


def kernel(query, key, value, Wq, bq, Wk, bk, Wv, bv):
    from concourse.bass_utils import run_bass_kernel_spmd

    nc = get_nc()
    in_maps = make_in_maps(query, key, value, Wq, bq, Wk, bk, Wv, bv)
    res = run_bass_kernel_spmd(nc, in_maps, list(range(B)))
    return postprocess(res.results, bv)



# revision 2
# speedup vs baseline: 22.9980x; 22.9980x over previous
"""Bass/Trainium2 kernel for nn_CrossAttentionLayer.

out = softmax((x_q Wq^T + bq)(x_k Wk^T + bk)^T) (x_v Wv^T + bv)

Sharding: data-parallel over batch B=8 across the 8 NeuronCores.
Exact math simplifications used:
  - bk drops out of softmax (adds a per-row constant to the logits).
  - bv is added on the host (softmax rows sum to 1, so attn @ (v0 + bv)
    = attn @ v0 + bv).
  - softmax normalization (divide by row-sum) commutes with the PV
    matmul, so the device returns the unnormalized PV product plus
    row-sums and the host divides.
Device layout: q and k projections are computed directly in transposed
[e, token] layout (lhsT = W^T chunks, rhs = x^T chunks), so no PE
transposes are needed anywhere; scores come out as [key, query] tiles
and the PV matmul consumes v in its natural [key, d] layout. Row-sums
are accumulated across key tiles on the (otherwise idle) Pool engine
and reduced over partitions with a single ones-vector matmul per query
block, keeping the PE free for the roofline matmuls.
"""

import sys

if "/opt/trn_rl_repo" not in sys.path:
    sys.path.insert(0, "/opt/trn_rl_repo")

import numpy as np

B = 8          # batch == number of cores
D = 1024       # model/latent dim
N = 2048       # tokens (queries == keys)
P = 128        # partitions
DC = D // P    # 8 chunks of the d/e axis
JT = N // P    # 16 key tiles
F = 512        # matmul moving free dim (fp32 max)
NB = N // F    # 4 query blocks

_CACHE = {}


def _build_nc():
    import concourse.bass as bass
    import concourse.mybir as mybir
    import concourse.tile as tile
    from concourse import bacc
    from contextlib import ExitStack

    f32 = mybir.dt.float32
    f32r = mybir.dt.float32r
    EXP = mybir.ActivationFunctionType.Exp
    IDENT = mybir.ActivationFunctionType.Identity

    nc = bacc.Bacc("TRN2", target_bir_lowering=False, debug=False, num_devices=B)

    xqt = nc.dram_tensor("xqt", [D, N], f32r, kind="ExternalInput").ap()
    xkt = nc.dram_tensor("xkt", [D, N], f32r, kind="ExternalInput").ap()
    xvt = nc.dram_tensor("xvt", [D, N], f32r, kind="ExternalInput").ap()
    wqt = nc.dram_tensor("wqt", [D, D], f32r, kind="ExternalInput").ap()
    wkt = nc.dram_tensor("wkt", [D, D], f32r, kind="ExternalInput").ap()
    wvt = nc.dram_tensor("wvt", [D, D], f32r, kind="ExternalInput").ap()
    bqt = nc.dram_tensor("bqt", [P, DC], f32, kind="ExternalInput").ap()

    v_int = nc.dram_tensor("v_int", [N, D], f32r).ap()
    qt_int = nc.dram_tensor("qt_int", [D, N], f32r).ap()

    acct = nc.dram_tensor("acct", [D, N], f32, kind="ExternalOutput").ap()
    rowsum = nc.dram_tensor("rowsum", [NB, F], f32, kind="ExternalOutput").ap()

    with ExitStack() as ctx:
        tc = ctx.enter_context(tile.TileContext(nc))
        big = ctx.enter_context(tc.tile_pool(name="big", bufs=2))
        ktp = ctx.enter_context(tc.tile_pool(name="ktp", bufs=1))
        xkq = ctx.enter_context(tc.tile_pool(name="xkq", bufs=2))
        xvp = ctx.enter_context(tc.tile_pool(name="xvp", bufs=3))
        vip = ctx.enter_context(tc.tile_pool(name="vip", bufs=2))
        stp = ctx.enter_context(tc.tile_pool(name="stp", bufs=3))
        rsp = ctx.enter_context(tc.tile_pool(name="rsp", bufs=2))
        cst = ctx.enter_context(tc.tile_pool(name="cst", bufs=1))
        psa = ctx.enter_context(tc.tile_pool(name="psa", bufs=3, space="PSUM"))
        psm = ctx.enter_context(tc.tile_pool(name="psm", bufs=2, space="PSUM"))
        psr = ctx.enter_context(tc.tile_pool(name="psr", bufs=1, space="PSUM"))

        ones_f = cst.tile([P, 1], f32, name="ones_f", tag="c_onesf")
        nc.vector.memset(ones_f, 1.0)
        ones = cst.tile([P, 1], f32r, name="ones", tag="c_ones")
        nc.vector.tensor_copy(ones, ones_f)
        bq_sb = cst.tile([P, DC], f32, name="bq_sb", tag="c_bq")

        kt_sb = ktp.tile([P, DC, N], f32r, name="kt_sb", tag="kt")

        xkr = xkt.rearrange("(c p) n -> p c n", p=P)
        xvr = xvt.rearrange("(c p) n -> p c n", p=P)
        xqr = xqt.rearrange("(c p) n -> p c n", p=P)
        qtr = qt_int.rearrange("(c p) n -> p c n", p=P)

        def load_w(dst, src):
            srcr = src.rearrange("(c p) e -> p c e", p=P)
            for c in range(DC):
                nc.sync.dma_start(out=dst[:, c, :], in_=srcr[:, c, :])

        def load_x(srcr, nb, name):
            # per-chunk DMAs so matmuls can start on chunk 0 before the
            # whole 2MB tile has landed
            xt = xkq.tile([P, DC, F], f32r, name=name, tag="xkq")
            for c in range(DC):
                nc.sync.dma_start(
                    out=xt[:, c, :], in_=srcr[:, c, nb * F:(nb + 1) * F]
                )
            return xt

        xt0 = load_x(xkr, 0, "xt_k")
        wk_sb = big.tile([P, DC, D], f32r, name="wk_sb", tag="big")
        load_w(wk_sb, wkt)
        wv_sb = big.tile([P, DC, D], f32r, name="wv_sb", tag="big")
        load_w(wv_sb, wvt)
        nc.sync.dma_start(out=bq_sb, in_=bqt)

        # ---- Phase K: kT[e, n] = (x_k Wk^T)^T directly, resident in SBUF ----
        for nb in range(NB):
            xt = xt0 if nb == 0 else load_x(xkr, nb, "xt_k")
            for e in range(DC):
                ps = psa.tile([P, F], f32, name="ps_k", tag="psa")
                for c in range(DC):
                    nc.tensor.matmul(
                        ps,
                        lhsT=wk_sb[:, c, e * P:(e + 1) * P],
                        rhs=xt[:, c, :],
                        start=(c == 0),
                        stop=(c == DC - 1),
                    )
                nc.scalar.copy(kt_sb[:, e, nb * F:(nb + 1) * F], ps)

        # ---- Phase V: v = x_v Wv^T (no bias) -> DRAM, natural [n, d] ----
        for jt in range(JT):
            xt = xvp.tile([P, DC, P], f32r, name="xt_v", tag="xv")
            nc.sync.dma_start(out=xt, in_=xvr[:, :, jt * P:(jt + 1) * P])
            for nd in range(2):
                ps = psa.tile([P, F], f32, name="ps_v", tag="psa")
                for c in range(DC):
                    nc.tensor.matmul(
                        ps,
                        lhsT=xt[:, c, :],
                        rhs=wv_sb[:, c, nd * F:(nd + 1) * F],
                        start=(c == 0),
                        stop=(c == DC - 1),
                    )
                st = stp.tile([P, F], f32r, name="st_v", tag="st")
                nc.vector.tensor_copy(st, ps)
                nc.sync.dma_start(
                    out=v_int[jt * P:(jt + 1) * P, nd * F:(nd + 1) * F], in_=st
                )

        # ---- Phase Q: qT[e, n] = (x_q Wq^T + bq)^T directly -> DRAM ----
        wq_sb = big.tile([P, DC, D], f32r, name="wq_sb", tag="big")
        load_w(wq_sb, wqt)
        for nb in range(NB):
            xt = load_x(xqr, nb, "xt_q")
            for e in range(DC):
                ps = psa.tile([P, F], f32, name="ps_q", tag="psa")
                for c in range(DC):
                    nc.tensor.matmul(
                        ps,
                        lhsT=wq_sb[:, c, e * P:(e + 1) * P],
                        rhs=xt[:, c, :],
                        start=(c == 0),
                        stop=(c == DC - 1),
                    )
                st = stp.tile([P, F], f32r, name="st_q", tag="st")
                nc.scalar.activation(st, ps, IDENT, bias=bq_sb[:, e:e + 1])
                nc.sync.dma_start(
                    out=qtr[:, e, nb * F:(nb + 1) * F], in_=st
                )

        # ---- Phase 2: per 512-query block: scores^T, exp, rowsum, PV ----
        vr = v_int.rearrange("(t p) d -> p t d", p=P)
        for t in range(NB):
            qb = xkq.tile([P, DC, F], f32r, name="qb", tag="xkq")
            for c in range(DC):
                nc.sync.dma_start(out=qb[:, c, :], in_=qtr[:, c, t * F:(t + 1) * F])
            ex = big.tile([P, JT, F], f32r, name="ex", tag="big")
            racc = rsp.tile([P, F], f32r, name="racc", tag="racc")
            for jt in range(JT):
                ps = psm.tile([P, F], f32, name="ps_s", tag="psm")
                for e in range(DC):
                    nc.tensor.matmul(
                        ps,
                        lhsT=kt_sb[:, e, jt * P:(jt + 1) * P],
                        rhs=qb[:, e, :],
                        start=(e == 0),
                        stop=(e == DC - 1),
                    )
                nc.scalar.activation(ex[:, jt, :], ps, EXP)
                # key-tile accumulation for the softmax denominator on the
                # idle Pool engine; partition reduction happens once below
                if jt == 0:
                    nc.gpsimd.tensor_copy(racc, ex[:, 0, :])
                else:
                    nc.gpsimd.tensor_add(racc, racc, ex[:, jt, :])
            rp = psr.tile([1, F], f32, name="rp", tag="psr")
            nc.tensor.matmul(
                rp, lhsT=ones, rhs=racc, start=True, stop=True,
                skip_group_check=True,
            )
            rs = stp.tile([1, F], f32, name="rs", tag="strs")
            nc.vector.tensor_copy(rs, rp)
            nc.sync.dma_start(out=rowsum[t:t + 1, :], in_=rs)
            for d in range(DC):
                vd = vip.tile([P, JT, P], f32r, name="vd", tag="vi")
                nc.sync.dma_start(out=vd[:, :8, :], in_=vr[:, :8, d * P:(d + 1) * P])
                nc.sync.dma_start(out=vd[:, 8:, :], in_=vr[:, 8:, d * P:(d + 1) * P])
                pv = psa.tile([P, F], f32, name="pv", tag="psa")
                for jt in range(JT):
                    nc.tensor.matmul(
                        pv,
                        lhsT=vd[:, jt, :],
                        rhs=ex[:, jt, :],
                        start=(jt == 0),
                        stop=(jt == JT - 1),
                    )
                ot = stp.tile([P, F], f32, name="ot", tag="st")
                nc.vector.tensor_copy(ot, pv)
                nc.sync.dma_start(
                    out=acct[d * P:(d + 1) * P, t * F:(t + 1) * F], in_=ot
                )

    nc.compile()
    return nc


def get_nc():
    if "nc" not in _CACHE:
        _CACHE["nc"] = _build_nc()
    return _CACHE["nc"]


def make_in_maps(query, key, value, Wq, bq, Wk, bk, Wv, bv):
    query = np.asarray(query, dtype=np.float32)
    key = np.asarray(key, dtype=np.float32)
    value = np.asarray(value, dtype=np.float32)
    wqt = np.ascontiguousarray(np.asarray(Wq, dtype=np.float32).T)
    wkt = np.ascontiguousarray(np.asarray(Wk, dtype=np.float32).T)
    wvt = np.ascontiguousarray(np.asarray(Wv, dtype=np.float32).T)
    bqt = np.ascontiguousarray(np.asarray(bq, dtype=np.float32).reshape(DC, P).T)
    in_maps = []
    for b in range(B):
        in_maps.append(
            {
                "xqt": np.ascontiguousarray(query[b].T),
                "xkt": np.ascontiguousarray(key[b].T),
                "xvt": np.ascontiguousarray(value[b].T),
                "wqt": wqt,
                "wkt": wkt,
                "wvt": wvt,
                "bqt": bqt,
            }
        )
    return in_maps


def postprocess(results, bv):
    bv = np.asarray(bv, dtype=np.float32)
    outs = []
    for b in range(B):
        acct = results[b]["acct"]              # [D, N] unnormalized (attn@v)^T
        rsum = results[b]["rowsum"].reshape(N)  # [N] softmax denominators
        outs.append(acct.T / rsum[:, None] + bv[None, :])
    return np.stack(outs).astype(np.float32)


def kernel(query, key, value, Wq, bq, Wk, bk, Wv, bv):
    from concourse.bass_utils import run_bass_kernel_spmd

    nc = get_nc()
    in_maps = make_in_maps(query, key, value, Wq, bq, Wk, bk, Wv, bv)
    res = run_bass_kernel_spmd(nc, in_maps, list(range(B)))
    return postprocess(res.results, bv)
